# revision 3
# baseline (speedup 1.0000x reference)
"""Trainium2 Bass kernel: AttentionPooling (attention-weighted global_add_pool).

Computes, for x [N, 256], sorted graph ids batch [N] (num_graphs=4096):
    h    = tanh(x @ W1 + b1)            # [N, 128]
    attn = h @ W2 + b2                  # [N, 1]
    out  = segment_sum(x * attn, batch) # [4096, 256]

v2 design (vs the fp16-compensated v1):
  - Pure fp16 inputs (x, W1, W2 as fp16; f32 PSUM accumulation). Host-checked
    rel err ~5e-4, well under the 2e-2 gate. Halves HBM traffic (32 MB/core)
    and removes 4 of the 10 matmul terms.
  - Transposes are REGULAR matmuls against a stationary identity
    (out = x_half.T @ I). PE-transpose-mode ops cost ~275 ns each (SBUF
    access latency dominated, don't keep HAM warm); a regular fp16 matmul
    streams N=128 rows at ~81 ns in a production stream and keeps the PE
    clock warm.
  - tanh output and the attention dot run in fp16 (th stationary, w2 fp16).
  - Optional reps>1 wraps the whole body in a hardware For_i loop: the NEFF
    re-executes the complete kernel (x re-read from HBM each iteration)
    reps times. Used by the harness to measure per-execution device time
    differentially through the high-overhead axon tunnel.

Per 128-row tile on-device:
  - xT halves via 2 regular matmuls (lhsT = x_half, rhs = I) -> f32 PSUM
  - PSUM -> SBUF fp16 copy (DVE/ACT alternating per group: single reader
    engine per PSUM buffer keeps every PE instruction within the
    2-engine sync-wait limit)
  - hT[a, n] = sum_d W1[d, a] xT[d, n]  (two fp16 K=128 matmuls, PSUM acc)
  - th = tanh(hT + b1) on ScalarE, fp16 out (bias per-partition: a)
  - attn[n, 1] = th.T @ W2 (fp16 matmul, free dim 1)
  - S[n, j] = (iota == rel[n]) * (attn[n] + b2)  (one fused DVE tensor_scalar
    into fp16; rel = batch - first_graph_of_window, host-precomputed)
  - acc[j, d] += S.T @ x_tile  (f32 PSUM accumulation across the window)
Window accumulators [32, 256] flush raw to DRAM; the host maps window slot
j -> graph g0[w] + j and sums across windows/cores (~8 MB, cheap).
"""

import math

import numpy as np

import concourse.bass as bass
import concourse.mybir as mybir
import concourse.tile as tile
from concourse import bacc, bass_utils

P = 128
D_IN = 256
D_ATT = 128
G_WIN = 32  # one-hot width = max graphs a window may span

N_NODES = 500_000
NUM_GRAPHS = 4096
N_CORES = 8
NODES_PER_CORE = N_NODES // N_CORES  # 62500
TILES_PER_CORE = math.ceil(NODES_PER_CORE / P)  # 489

F32 = mybir.dt.float32
F16 = mybir.dt.float16


def build_program(n_tiles: int, win_tiles: int, b2: float, reps: int = 1):
    """Build the single-core Bass program (same NEFF runs SPMD on all cores).

    reps > 1 wraps the body in a hardware For_i loop for differential
    device-time measurement; every iteration re-reads x from HBM and
    rewrites the full output."""
    assert n_tiles % win_tiles == 0, "pad tiles to a whole number of windows"
    n_wins = n_tiles // win_tiles
    nc = bacc.Bacc(trn_type="TRN2", target_bir_lowering=False, debug=False,
                   num_devices=N_CORES)

    n_const = 1 + G_WIN + n_tiles                 # b1 | iota | relT
    n_const16 = 2 * D_ATT + P + 1                 # W1 halves | idn | w2
    # x16: per window [128, win_tiles*256] fp16, host-swizzled so each
    # window's DMA is partition-contiguous (8 KB/partition, 128 descriptors)
    x_d = nc.dram_tensor("x16", [n_wins * P, win_tiles * D_IN], F16,
                         kind="ExternalInput").ap()
    cst_d = nc.dram_tensor("cst", [P, n_const], F32, kind="ExternalInput").ap()
    c16_d = nc.dram_tensor("cst16", [P, n_const16], F16,
                           kind="ExternalInput").ap()
    out_d = nc.dram_tensor("out", [n_wins * G_WIN, D_IN], F32,
                           kind="ExternalOutput").ap()

    with tile.TileContext(nc) as tc:
        with (
            tc.tile_pool(name="consts", bufs=1) as cpool,
            tc.tile_pool(name="xin", bufs=3) as xpool,
            tc.tile_pool(name="xtsb", bufs=3) as xtpool,
            tc.tile_pool(name="thsb", bufs=3) as thpool,
            tc.tile_pool(name="attnsb", bufs=3) as apool,
            tc.tile_pool(name="ssb", bufs=4) as spool,
            tc.tile_pool(name="outsb", bufs=2) as opool,
            tc.tile_pool(name="xtps", bufs=2, space="PSUM") as xtps_pool,
            tc.tile_pool(name="htps", bufs=2, space="PSUM") as htps_pool,
            tc.tile_pool(name="atps", bufs=2, space="PSUM") as atps_pool,
            tc.tile_pool(name="accps", bufs=2, space="PSUM") as accps_pool,
        ):
            # constants: loaded once, before the (optional) repeat loop
            cst_sb = cpool.tile([P, n_const], F32, name="cst_sb")
            nc.sync.dma_start(out=cst_sb, in_=cst_d)
            o = 0
            b1_sb = cst_sb[:, o:o + 1]; o += 1
            iota_sb = cst_sb[:, o:o + G_WIN]; o += G_WIN
            relT_sb = cst_sb[:, o:o + n_tiles]; o += n_tiles

            c16_sb = cpool.tile([P, n_const16], F16, name="c16_sb")
            nc.sync.dma_start(out=c16_sb, in_=c16_d)
            w1h = [c16_sb[:, 0:P], c16_sb[:, P:2 * P]]
            idn_sb = c16_sb[:, 2 * P:3 * P]
            w2_sb = c16_sb[:, 3 * P:3 * P + 1]

            def body():
                for w in range(n_wins):
                    t0 = w * win_tiles
                    wt = win_tiles

                    x_chunk = xpool.tile([P, wt * D_IN], F16, name="x_chunk",
                                         tag="x_chunk")
                    nc.sync.dma_start(
                        out=x_chunk, in_=x_d[w * P:(w + 1) * P, :])

                    acc_ps = accps_pool.tile([G_WIN, D_IN], F32, name="acc_ps",
                                             tag="acc_ps")

                    groups = [tuple(range(g, min(g + 2, wt)))
                              for g in range(0, wt, 2)]
                    for gi, grp in enumerate(groups):
                        ng = len(grp)
                        # --- xT via regular matmul: xT_half = x_half.T @ I ---
                        xt_ps = xtps_pool.tile([P, ng * D_IN], F32,
                                               name="xt_ps", tag="xt_ps")
                        for i, lt in enumerate(grp):
                            for q in range(2):
                                nc.tensor.matmul(
                                    xt_ps[:, i * D_IN + q * P:
                                          i * D_IN + (q + 1) * P],
                                    x_chunk[:, lt * D_IN + q * P:
                                            lt * D_IN + (q + 1) * P],
                                    idn_sb, start=True, stop=True)
                        # PSUM -> SBUF fp16, always DVE (single reader per
                        # buffer keeps PE within its 2-engine wait limit;
                        # warm ACT copies are ~2.1x slower than DVE and ACT
                        # is reserved for tanh)
                        xt_sb = xtpool.tile([P, ng * D_IN], F16, name="xt_sb",
                                            tag="xt_sb")
                        nc.vector.tensor_copy(xt_sb, xt_ps[:, 0:ng * D_IN])

                        # --- hT accumulated over the two d-halves ---
                        ht_ps = htps_pool.tile([P, ng * D_ATT], F32,
                                               name="ht_ps", tag="ht_ps")
                        xt4 = xt_sb.rearrange("p (t h n) -> p t h n",
                                              t=ng, h=2)
                        ht3 = ht_ps.rearrange("p (t n) -> p t n", t=ng)
                        nc.tensor.matmul(ht3, w1h[0], xt4[:, :, 0, :],
                                         start=True, stop=False)
                        nc.tensor.matmul(ht3, w1h[1], xt4[:, :, 1, :],
                                         start=False, stop=True)

                        # --- th = tanh(hT + b1), fp16 out ---
                        th_sb = thpool.tile([P, ng * D_ATT], F16, name="th_sb",
                                            tag="th_sb")
                        nc.scalar.activation(th_sb, ht_ps[:, 0:ng * D_ATT],
                                             mybir.ActivationFunctionType.Tanh,
                                             bias=b1_sb, scale=1.0)

                        # --- attn[n] = th.T @ W2 (fp16, free dim 1) ---
                        at_ps = atps_pool.tile([P, ng], F32, name="at_ps",
                                               tag="at_ps")
                        for i in range(ng):
                            nc.tensor.matmul(
                                at_ps[:, i:i + 1],
                                th_sb[:, i * D_ATT:(i + 1) * D_ATT],
                                w2_sb, start=True, stop=True)
                        at_sb = apool.tile([P, ng], F32, name="at_sb",
                                           tag="at_sb")
                        nc.vector.tensor_scalar_add(at_sb, at_ps[:, 0:ng],
                                                    float(b2))

                        # --- S = (iota == rel) * attn' ; acc += S.T @ x ---
                        # S-build on GpSimd (SBUF-only inputs): unloads DVE,
                        # whose span is dominated by the xt PSUM reads.
                        for i, lt in enumerate(grp):
                            gt = t0 + lt
                            s_sb = spool.tile([P, G_WIN], F16, name="s_sb",
                                              tag="s_sb")
                            nc.gpsimd.tensor_scalar(
                                s_sb, iota_sb, relT_sb[:, gt:gt + 1],
                                at_sb[:, i:i + 1],
                                mybir.AluOpType.is_equal, mybir.AluOpType.mult)
                            nc.tensor.matmul(
                                acc_ps, s_sb,
                                x_chunk[:, lt * D_IN:(lt + 1) * D_IN],
                                start=(lt == 0), stop=(lt == wt - 1))

                    # --- flush window accumulator (DVE) ---
                    out_sb = opool.tile([G_WIN, D_IN], F32, name="out_sb",
                                        tag="out_sb")
                    nc.vector.tensor_copy(out_sb, acc_ps)
                    nc.sync.dma_start(
                        out=out_d[w * G_WIN:(w + 1) * G_WIN, :], in_=out_sb)

            if reps == 1:
                body()
            else:
                with tc.For_i(0, reps, 1):
                    body()

    nc.compile()
    return nc


def choose_win_tiles(batch_slices, n_tiles):
    """Pick the biggest window size (in tiles) such that every window of
    every core spans < G_WIN distinct graphs (batch is sorted, so the span
    is last - first + 1)."""
    for wt in (16, 8, 4, 2, 1):
        ok = True
        for bc in batch_slices:
            nn = len(bc)
            for s in range(0, nn, wt * P):
                e = min(nn, s + wt * P)
                if bc[e - 1] - bc[s] + 1 > G_WIN - 1:
                    ok = False
                    break
            if not ok:
                break
        if ok:
            return wt
    return 1


def prep_core(x_real, batch_real, n_tiles, win_tiles):
    """Pad one core's slice to n_tiles*128 nodes (whole windows), cast to
    fp16, swizzle per window to a partition-contiguous layout, and build
    relT + g0s.

    Returns (x_sw [n_wins*128, win_tiles*256] f16, relT [128, n_tiles] f32,
    g0s). Padded nodes get rel = -1 so they never match the one-hot iota.
    x_sw[w*128 + p, t*256:(t+1)*256] = x[(w*win_tiles + t)*128 + p].
    """
    assert n_tiles % win_tiles == 0
    npad = n_tiles * P
    n_real = x_real.shape[0]
    assert n_real <= npad
    x_pad = np.zeros((npad, D_IN), dtype=np.float16)
    x_pad[:n_real] = x_real.astype(np.float16)
    b = np.full(npad, -1, dtype=np.int64)
    b[:n_real] = batch_real

    n_wins = n_tiles // win_tiles
    x_sw = np.ascontiguousarray(
        x_pad.reshape(n_wins, win_tiles, P, D_IN).transpose(0, 2, 1, 3)
    ).reshape(n_wins * P, win_tiles * D_IN)

    rel = np.full(npad, -1.0, dtype=np.float32)
    g0s = np.zeros(n_wins, dtype=np.int64)
    for w in range(n_wins):
        s = w * win_tiles * P
        e = (w + 1) * win_tiles * P
        seg = b[s:e]
        realm = seg >= 0
        g0 = int(seg[realm][0]) if realm.any() else 0
        g0s[w] = g0
        rw = (seg - g0).astype(np.float32)
        rw[~realm] = -1.0
        assert rw.max() < G_WIN, (
            f"window spans too many graphs: {rw.max()} >= {G_WIN}")
        rel[s:e] = rw
    relT = np.ascontiguousarray(rel.reshape(n_tiles, P).T)
    return x_sw, relT, g0s


def make_consts(W1, b1, W2):
    """Returns (cst_f32 [128, 33], cst16 [128, 385])."""
    W1 = np.asarray(W1, dtype=np.float32)
    cst = np.ascontiguousarray(np.concatenate([
        np.asarray(b1, np.float32).reshape(P, 1),
        np.broadcast_to(np.arange(G_WIN, dtype=np.float32), (P, G_WIN)),
    ], axis=1))
    w1h = W1.astype(np.float16)
    cst16 = np.ascontiguousarray(np.concatenate([
        w1h[0:P, :], w1h[P:2 * P, :],
        np.eye(P, dtype=np.float16),
        np.asarray(W2, np.float16).reshape(P, 1),
    ], axis=1))
    return cst, cst16


def postprocess(raws, g0s_per_core, num_graphs):
    """raws: per-core [n_wins*G_WIN, D_IN] raw window sums -> [G, D_IN]."""
    out = np.zeros((num_graphs, D_IN), dtype=np.float64)
    for raw, g0s in zip(raws, g0s_per_core):
        raw3 = raw.reshape(-1, G_WIN, D_IN)
        for w, g0 in enumerate(g0s):
            width = min(G_WIN, num_graphs - int(g0))
            out[g0:g0 + width] += raw3[w, :width]
    return out.astype(np.float32)


def prepare(x, batch, num_graphs, W1, b1, W2, b2, reps=1):
    """Host-side prep: shard, window metadata, and the Bass program.

    Returns (nc, in_maps, g0s_per_core, num_graphs).
    """
    x = np.asarray(x, dtype=np.float32)
    batch = np.asarray(batch).astype(np.int64)
    num_graphs = int(num_graphs)
    W1 = np.asarray(W1, dtype=np.float32)
    b1 = np.asarray(b1, dtype=np.float32)
    W2 = np.asarray(W2, dtype=np.float32)
    b2f = float(np.asarray(b2).reshape(-1)[0])

    n = x.shape[0]
    assert n == N_NODES and x.shape[1] == D_IN
    assert np.all(np.diff(batch) >= 0), "batch must be sorted"

    bounds = [(c * NODES_PER_CORE,
               min(n, (c + 1) * NODES_PER_CORE) if c < N_CORES - 1 else n)
              for c in range(N_CORES)]

    wt = choose_win_tiles([batch[s:e] for s, e in bounds], TILES_PER_CORE)
    n_tiles_pad = math.ceil(TILES_PER_CORE / wt) * wt

    cbase, cst16 = make_consts(W1, b1, W2)
    in_maps = []
    g0s_per_core = []
    for s, e in bounds:
        x_sw, relT, g0s = prep_core(x[s:e], batch[s:e], n_tiles_pad, wt)
        cst = np.ascontiguousarray(np.concatenate([cbase, relT], axis=1))
        in_maps.append({"x16": x_sw, "cst": cst, "cst16": cst16})
        g0s_per_core.append(g0s)
    nc = build_program(n_tiles_pad, wt, b2f, reps=reps)
    return nc, in_maps, g0s_per_core, num_graphs


def kernel(x, batch, num_graphs, W1, b1, W2, b2):
    nc, in_maps, g0s_per_core, num_graphs = prepare(
        x, batch, num_graphs, W1, b1, W2, b2)
    res = bass_utils.run_bass_kernel_spmd(
        nc, in_maps, core_ids=list(range(N_CORES)))
    raws = [r["out"] for r in res.results]
    return postprocess(raws, g0s_per_core, num_graphs)


# revision 4
# speedup vs baseline: 2.1167x; 2.1167x over previous
"""Trainium2 Bass kernel: AttentionPooling (attention-weighted global_add_pool).

Computes, for x [N, 256], sorted graph ids batch [N] (num_graphs=4096):
    h    = tanh(x @ W1 + b1)            # [N, 128]
    attn = h @ W2 + b2                  # [N, 1]
    out  = segment_sum(x * attn, batch) # [4096, 256]

v2 design (vs the fp16-compensated v1):
  - Pure fp16 inputs (x, W1, W2 as fp16; f32 PSUM accumulation). Host-checked
    rel err ~5e-4, well under the 2e-2 gate. Halves HBM traffic (32 MB/core)
    and removes 4 of the 10 matmul terms.
  - Transposes are REGULAR matmuls against a stationary identity
    (out = x_half.T @ I). PE-transpose-mode ops cost ~275 ns each (SBUF
    access latency dominated, don't keep HAM warm); a regular fp16 matmul
    streams N=128 rows at ~81 ns in a production stream and keeps the PE
    clock warm.
  - tanh output and the attention dot run in fp16 (th stationary, w2 fp16).
  - Optional reps>1 wraps the whole body in a hardware For_i loop: the NEFF
    re-executes the complete kernel (x re-read from HBM each iteration)
    reps times. Used by the harness to measure per-execution device time
    differentially through the high-overhead axon tunnel.

Per 128-row tile on-device:
  - xT halves via 2 regular matmuls (lhsT = x_half, rhs = I) -> f32 PSUM
  - PSUM -> SBUF fp16 copy (DVE/ACT alternating per group: single reader
    engine per PSUM buffer keeps every PE instruction within the
    2-engine sync-wait limit)
  - hT[a, n] = sum_d W1[d, a] xT[d, n]  (two fp16 K=128 matmuls, PSUM acc)
  - th = tanh(hT + b1) on ScalarE, fp16 out (bias per-partition: a)
  - attn[n, 1] = th.T @ W2 (fp16 matmul, free dim 1)
  - S[n, j] = (iota == rel[n]) * (attn[n] + b2)  (one fused DVE tensor_scalar
    into fp16; rel = batch - first_graph_of_window, host-precomputed)
  - acc[j, d] += S.T @ x_tile  (f32 PSUM accumulation across the window)
Window accumulators [32, 256] flush raw to DRAM; the host maps window slot
j -> graph g0[w] + j and sums across windows/cores (~8 MB, cheap).
"""

import math

import numpy as np

import concourse.bass as bass
import concourse.mybir as mybir
import concourse.tile as tile
from concourse import bacc, bass_utils

P = 128
D_IN = 256
D_ATT = 128
G_WIN = 32  # one-hot width = max graphs a window may span

N_NODES = 500_000
NUM_GRAPHS = 4096
N_CORES = 8
NODES_PER_CORE = N_NODES // N_CORES  # 62500
TILES_PER_CORE = math.ceil(NODES_PER_CORE / P)  # 489

F32 = mybir.dt.float32
F16 = mybir.dt.float16


def build_program(n_tiles: int, win_tiles: int, b2: float, reps: int = 1,
                  stages: str = "all"):
    """Build the single-core Bass program (same NEFF runs SPMD on all cores).

    reps > 1 wraps the body in a hardware For_i loop for differential
    device-time measurement; every iteration re-reads x from HBM and
    rewrites the full output.

    stages: "all" (the real kernel) | "noattn" (skip transpose/hT/tanh/attn;
    S = bare one-hot) | "nopool" (only one pool matmul per window) —
    timing-only ablations for bottleneck attribution; their outputs are
    numerically wrong."""
    assert n_tiles % win_tiles == 0, "pad tiles to a whole number of windows"
    n_wins = n_tiles // win_tiles
    nc = bacc.Bacc(trn_type="TRN2", target_bir_lowering=False, debug=False,
                   num_devices=N_CORES)

    n_const = 1 + G_WIN + n_tiles                 # b1 | iota | relT
    n_const16 = 2 * D_ATT + P + 1                 # W1 halves | idn | w2
    # x16: per window [128, win_tiles*256] fp16, host-swizzled so each
    # window's DMA is partition-contiguous (8 KB/partition, 128 descriptors)
    x_d = nc.dram_tensor("x16", [n_wins * P, win_tiles * D_IN], F16,
                         kind="ExternalInput").ap()
    cst_d = nc.dram_tensor("cst", [P, n_const], F32, kind="ExternalInput").ap()
    c16_d = nc.dram_tensor("cst16", [P, n_const16], F16,
                           kind="ExternalInput").ap()
    out_d = nc.dram_tensor("out", [n_wins * G_WIN, D_IN], F32,
                           kind="ExternalOutput").ap()

    with tile.TileContext(nc) as tc:
        with (
            tc.tile_pool(name="consts", bufs=1) as cpool,
            tc.tile_pool(name="xin", bufs=3) as xpool,
            tc.tile_pool(name="xtsb", bufs=3) as xtpool,
            tc.tile_pool(name="thsb", bufs=3) as thpool,
            tc.tile_pool(name="attnsb", bufs=3) as apool,
            tc.tile_pool(name="ssb", bufs=4) as spool,
            tc.tile_pool(name="outsb", bufs=2) as opool,
            tc.tile_pool(name="xtps", bufs=2, space="PSUM") as xtps_pool,
            tc.tile_pool(name="htps", bufs=2, space="PSUM") as htps_pool,
            tc.tile_pool(name="atps", bufs=2, space="PSUM") as atps_pool,
            tc.tile_pool(name="accps", bufs=2, space="PSUM") as accps_pool,
        ):
            # constants: loaded once, before the (optional) repeat loop
            cst_sb = cpool.tile([P, n_const], F32, name="cst_sb")
            nc.sync.dma_start(out=cst_sb, in_=cst_d)
            o = 0
            b1_sb = cst_sb[:, o:o + 1]; o += 1
            iota_sb = cst_sb[:, o:o + G_WIN]; o += G_WIN
            relT_sb = cst_sb[:, o:o + n_tiles]; o += n_tiles

            c16_sb = cpool.tile([P, n_const16], F16, name="c16_sb")
            nc.sync.dma_start(out=c16_sb, in_=c16_d)
            w1h = [c16_sb[:, 0:P], c16_sb[:, P:2 * P]]
            idn_sb = c16_sb[:, 2 * P:3 * P]
            w2_sb = c16_sb[:, 3 * P:3 * P + 1]

            def body():
                for w in range(n_wins):
                    t0 = w * win_tiles
                    wt = win_tiles

                    x_chunk = xpool.tile([P, wt * D_IN], F16, name="x_chunk",
                                         tag="x_chunk")
                    nc.sync.dma_start(
                        out=x_chunk, in_=x_d[w * P:(w + 1) * P, :])

                    acc_ps = accps_pool.tile([G_WIN, D_IN], F32, name="acc_ps",
                                             tag="acc_ps")

                    groups = [tuple(range(g, min(g + 2, wt)))
                              for g in range(0, wt, 2)]
                    for gi, grp in enumerate(groups):
                        ng = len(grp)
                        if stages == "noattn":
                            for i, lt in enumerate(grp):
                                gt = t0 + lt
                                s_sb = spool.tile([P, G_WIN], F16,
                                                  name="s_sb", tag="s_sb")
                                nc.vector.tensor_scalar(
                                    s_sb, iota_sb, relT_sb[:, gt:gt + 1],
                                    None, mybir.AluOpType.is_equal, None)
                                nc.tensor.matmul(
                                    acc_ps, s_sb,
                                    x_chunk[:, lt * D_IN:(lt + 1) * D_IN],
                                    start=(lt == 0), stop=(lt == wt - 1))
                            continue
                        # --- xT via regular matmul: xT_half = x_half.T @ I ---
                        xt_ps = xtps_pool.tile([P, ng * D_IN], F32,
                                               name="xt_ps", tag="xt_ps")
                        for i, lt in enumerate(grp):
                            for q in range(2):
                                nc.tensor.matmul(
                                    xt_ps[:, i * D_IN + q * P:
                                          i * D_IN + (q + 1) * P],
                                    x_chunk[:, lt * D_IN + q * P:
                                            lt * D_IN + (q + 1) * P],
                                    idn_sb, start=True, stop=True)
                        # PSUM -> SBUF fp16, always DVE (single reader per
                        # buffer keeps PE within its 2-engine wait limit;
                        # warm ACT copies are ~2.1x slower than DVE and ACT
                        # is reserved for tanh)
                        xt_sb = xtpool.tile([P, ng * D_IN], F16, name="xt_sb",
                                            tag="xt_sb")
                        nc.vector.tensor_copy(xt_sb, xt_ps[:, 0:ng * D_IN])

                        # --- hT accumulated over the two d-halves ---
                        ht_ps = htps_pool.tile([P, ng * D_ATT], F32,
                                               name="ht_ps", tag="ht_ps")
                        xt4 = xt_sb.rearrange("p (t h n) -> p t h n",
                                              t=ng, h=2)
                        ht3 = ht_ps.rearrange("p (t n) -> p t n", t=ng)
                        nc.tensor.matmul(ht3, w1h[0], xt4[:, :, 0, :],
                                         start=True, stop=False)
                        nc.tensor.matmul(ht3, w1h[1], xt4[:, :, 1, :],
                                         start=False, stop=True)

                        # --- th = tanh(hT + b1), fp16 out ---
                        th_sb = thpool.tile([P, ng * D_ATT], F16, name="th_sb",
                                            tag="th_sb")
                        nc.scalar.activation(th_sb, ht_ps[:, 0:ng * D_ATT],
                                             mybir.ActivationFunctionType.Tanh,
                                             bias=b1_sb, scale=1.0)

                        # --- attn[n] = th.T @ W2 (fp16, free dim 1) ---
                        at_ps = atps_pool.tile([P, ng], F32, name="at_ps",
                                               tag="at_ps")
                        for i in range(ng):
                            nc.tensor.matmul(
                                at_ps[:, i:i + 1],
                                th_sb[:, i * D_ATT:(i + 1) * D_ATT],
                                w2_sb, start=True, stop=True)
                        at_sb = apool.tile([P, ng], F32, name="at_sb",
                                           tag="at_sb")
                        nc.vector.tensor_scalar_add(at_sb, at_ps[:, 0:ng],
                                                    float(b2))

                        # --- S = (iota == rel) * attn' ; acc += S.T @ x ---
                        # (S-build stays on DVE: GpSimd shares an SBUF port
                        # with DVE and its per-instruction overhead regressed
                        # the kernel ~2x when tried.)
                        for i, lt in enumerate(grp):
                            gt = t0 + lt
                            s_sb = spool.tile([P, G_WIN], F16, name="s_sb",
                                              tag="s_sb")
                            nc.vector.tensor_scalar(
                                s_sb, iota_sb, relT_sb[:, gt:gt + 1],
                                at_sb[:, i:i + 1],
                                mybir.AluOpType.is_equal, mybir.AluOpType.mult)
                            nc.tensor.matmul(
                                acc_ps, s_sb,
                                x_chunk[:, lt * D_IN:(lt + 1) * D_IN],
                                start=(lt == 0), stop=(lt == wt - 1))

                    # --- flush window accumulator (DVE) ---
                    out_sb = opool.tile([G_WIN, D_IN], F32, name="out_sb",
                                        tag="out_sb")
                    nc.vector.tensor_copy(out_sb, acc_ps)
                    nc.sync.dma_start(
                        out=out_d[w * G_WIN:(w + 1) * G_WIN, :], in_=out_sb)

            if reps == 1:
                body()
            else:
                with tc.For_i(0, reps, 1):
                    body()

    nc.compile()
    return nc


def choose_win_tiles(batch_slices, n_tiles):
    """Pick the biggest window size (in tiles) such that every window of
    every core spans < G_WIN distinct graphs (batch is sorted, so the span
    is last - first + 1)."""
    for wt in (16, 8, 4, 2, 1):
        ok = True
        for bc in batch_slices:
            nn = len(bc)
            for s in range(0, nn, wt * P):
                e = min(nn, s + wt * P)
                if bc[e - 1] - bc[s] + 1 > G_WIN - 1:
                    ok = False
                    break
            if not ok:
                break
        if ok:
            return wt
    return 1


def prep_core(x_real, batch_real, n_tiles, win_tiles):
    """Pad one core's slice to n_tiles*128 nodes (whole windows), cast to
    fp16, swizzle per window to a partition-contiguous layout, and build
    relT + g0s.

    Returns (x_sw [n_wins*128, win_tiles*256] f16, relT [128, n_tiles] f32,
    g0s). Padded nodes get rel = -1 so they never match the one-hot iota.
    x_sw[w*128 + p, t*256:(t+1)*256] = x[(w*win_tiles + t)*128 + p].
    """
    assert n_tiles % win_tiles == 0
    npad = n_tiles * P
    n_real = x_real.shape[0]
    assert n_real <= npad
    x_pad = np.zeros((npad, D_IN), dtype=np.float16)
    x_pad[:n_real] = x_real.astype(np.float16)
    b = np.full(npad, -1, dtype=np.int64)
    b[:n_real] = batch_real

    n_wins = n_tiles // win_tiles
    x_sw = np.ascontiguousarray(
        x_pad.reshape(n_wins, win_tiles, P, D_IN).transpose(0, 2, 1, 3)
    ).reshape(n_wins * P, win_tiles * D_IN)

    rel = np.full(npad, -1.0, dtype=np.float32)
    g0s = np.zeros(n_wins, dtype=np.int64)
    for w in range(n_wins):
        s = w * win_tiles * P
        e = (w + 1) * win_tiles * P
        seg = b[s:e]
        realm = seg >= 0
        g0 = int(seg[realm][0]) if realm.any() else 0
        g0s[w] = g0
        rw = (seg - g0).astype(np.float32)
        rw[~realm] = -1.0
        assert rw.max() < G_WIN, (
            f"window spans too many graphs: {rw.max()} >= {G_WIN}")
        rel[s:e] = rw
    relT = np.ascontiguousarray(rel.reshape(n_tiles, P).T)
    return x_sw, relT, g0s


def make_consts(W1, b1, W2):
    """Returns (cst_f32 [128, 33], cst16 [128, 385])."""
    W1 = np.asarray(W1, dtype=np.float32)
    cst = np.ascontiguousarray(np.concatenate([
        np.asarray(b1, np.float32).reshape(P, 1),
        np.broadcast_to(np.arange(G_WIN, dtype=np.float32), (P, G_WIN)),
    ], axis=1))
    w1h = W1.astype(np.float16)
    cst16 = np.ascontiguousarray(np.concatenate([
        w1h[0:P, :], w1h[P:2 * P, :],
        np.eye(P, dtype=np.float16),
        np.asarray(W2, np.float16).reshape(P, 1),
    ], axis=1))
    return cst, cst16


def postprocess(raws, g0s_per_core, num_graphs):
    """raws: per-core [n_wins*G_WIN, D_IN] raw window sums -> [G, D_IN]."""
    out = np.zeros((num_graphs, D_IN), dtype=np.float64)
    for raw, g0s in zip(raws, g0s_per_core):
        raw3 = raw.reshape(-1, G_WIN, D_IN)
        for w, g0 in enumerate(g0s):
            width = min(G_WIN, num_graphs - int(g0))
            out[g0:g0 + width] += raw3[w, :width]
    return out.astype(np.float32)


def prepare(x, batch, num_graphs, W1, b1, W2, b2, reps=1):
    """Host-side prep: shard, window metadata, and the Bass program.

    Returns (nc, in_maps, g0s_per_core, num_graphs).
    """
    x = np.asarray(x, dtype=np.float32)
    batch = np.asarray(batch).astype(np.int64)
    num_graphs = int(num_graphs)
    W1 = np.asarray(W1, dtype=np.float32)
    b1 = np.asarray(b1, dtype=np.float32)
    W2 = np.asarray(W2, dtype=np.float32)
    b2f = float(np.asarray(b2).reshape(-1)[0])

    n = x.shape[0]
    assert n == N_NODES and x.shape[1] == D_IN
    assert np.all(np.diff(batch) >= 0), "batch must be sorted"

    bounds = [(c * NODES_PER_CORE,
               min(n, (c + 1) * NODES_PER_CORE) if c < N_CORES - 1 else n)
              for c in range(N_CORES)]

    wt = choose_win_tiles([batch[s:e] for s, e in bounds], TILES_PER_CORE)
    n_tiles_pad = math.ceil(TILES_PER_CORE / wt) * wt

    cbase, cst16 = make_consts(W1, b1, W2)
    in_maps = []
    g0s_per_core = []
    for s, e in bounds:
        x_sw, relT, g0s = prep_core(x[s:e], batch[s:e], n_tiles_pad, wt)
        cst = np.ascontiguousarray(np.concatenate([cbase, relT], axis=1))
        in_maps.append({"x16": x_sw, "cst": cst, "cst16": cst16})
        g0s_per_core.append(g0s)
    nc = build_program(n_tiles_pad, wt, b2f, reps=reps)
    return nc, in_maps, g0s_per_core, num_graphs


def kernel(x, batch, num_graphs, W1, b1, W2, b2):
    nc, in_maps, g0s_per_core, num_graphs = prepare(
        x, batch, num_graphs, W1, b1, W2, b2)
    res = bass_utils.run_bass_kernel_spmd(
        nc, in_maps, core_ids=list(range(N_CORES)))
    raws = [r["out"] for r in res.results]
    return postprocess(raws, g0s_per_core, num_graphs)


# revision 5
# speedup vs baseline: 2.1679x; 1.0242x over previous
"""Trainium2 Bass kernel: AttentionPooling (attention-weighted global_add_pool).

Computes, for x [N, 256], sorted graph ids batch [N] (num_graphs=4096):
    h    = tanh(x @ W1 + b1)            # [N, 128]
    attn = h @ W2 + b2                  # [N, 1]
    out  = segment_sum(x * attn, batch) # [4096, 256]

v2 design (vs the fp16-compensated v1):
  - Pure fp16 inputs (x, W1, W2 as fp16; f32 PSUM accumulation). Host-checked
    rel err ~5e-4, well under the 2e-2 gate. Halves HBM traffic (32 MB/core)
    and removes 4 of the 10 matmul terms.
  - Transposes are REGULAR matmuls against a stationary identity
    (out = x_half.T @ I). PE-transpose-mode ops cost ~275 ns each (SBUF
    access latency dominated, don't keep HAM warm); a regular fp16 matmul
    streams N=128 rows at ~81 ns in a production stream and keeps the PE
    clock warm.
  - tanh output and the attention dot run in fp16 (th stationary, w2 fp16).
  - Optional reps>1 wraps the whole body in a hardware For_i loop: the NEFF
    re-executes the complete kernel (x re-read from HBM each iteration)
    reps times. Used by the harness to measure per-execution device time
    differentially through the high-overhead axon tunnel.

Per 128-row tile on-device:
  - xT halves via 2 regular matmuls (lhsT = x_half, rhs = I) -> f32 PSUM
  - PSUM -> SBUF fp16 copy (DVE/ACT alternating per group: single reader
    engine per PSUM buffer keeps every PE instruction within the
    2-engine sync-wait limit)
  - hT[a, n] = sum_d W1[d, a] xT[d, n]  (two fp16 K=128 matmuls, PSUM acc)
  - th = tanh(hT + b1) on ScalarE, fp16 out (bias per-partition: a)
  - attn[n, 1] = th.T @ W2 (fp16 matmul, free dim 1)
  - S[n, j] = (iota == rel[n]) * (attn[n] + b2)  (one fused DVE tensor_scalar
    into fp16; rel = batch - first_graph_of_window, host-precomputed)
  - acc[j, d] += S.T @ x_tile  (f32 PSUM accumulation across the window)
Window accumulators [32, 256] flush raw to DRAM; the host maps window slot
j -> graph g0[w] + j and sums across windows/cores (~8 MB, cheap).
"""

import math

import numpy as np

import concourse.bass as bass
import concourse.mybir as mybir
import concourse.tile as tile
from concourse import bacc, bass_utils

P = 128
D_IN = 256
D_ATT = 128
G_WIN = 32  # one-hot width = max graphs a window may span

N_NODES = 500_000
NUM_GRAPHS = 4096
N_CORES = 8
NODES_PER_CORE = N_NODES // N_CORES  # 62500
TILES_PER_CORE = math.ceil(NODES_PER_CORE / P)  # 489

F32 = mybir.dt.float32
F16 = mybir.dt.float16


def build_program(n_tiles: int, win_tiles: int, b2: float, reps: int = 1,
                  stages: str = "all"):
    """Build the single-core Bass program (same NEFF runs SPMD on all cores).

    reps > 1 wraps the body in a hardware For_i loop for differential
    device-time measurement; every iteration re-reads x from HBM and
    rewrites the full output.

    stages: "all" (the real kernel) | "noattn" (skip transpose/hT/tanh/attn;
    S = bare one-hot) | "nopool" (only one pool matmul per window) —
    timing-only ablations for bottleneck attribution; their outputs are
    numerically wrong."""
    assert n_tiles % win_tiles == 0, "pad tiles to a whole number of windows"
    n_wins = n_tiles // win_tiles
    nc = bacc.Bacc(trn_type="TRN2", target_bir_lowering=False, debug=False,
                   num_devices=N_CORES)

    n_const = 1 + G_WIN + n_tiles                 # b1 | iota | relT
    n_const16 = 2 * D_ATT + P + 1                 # W1 halves | idn | w2
    # x16: per window [128, win_tiles*256] fp16, host-swizzled so each
    # window's DMA is partition-contiguous (8 KB/partition, 128 descriptors)
    x_d = nc.dram_tensor("x16", [n_wins * P, win_tiles * D_IN], F16,
                         kind="ExternalInput").ap()
    cst_d = nc.dram_tensor("cst", [P, n_const], F32, kind="ExternalInput").ap()
    c16_d = nc.dram_tensor("cst16", [P, n_const16], F16,
                           kind="ExternalInput").ap()
    out_d = nc.dram_tensor("out", [n_wins * G_WIN, D_IN], F32,
                           kind="ExternalOutput").ap()

    with tile.TileContext(nc) as tc:
        with (
            tc.tile_pool(name="consts", bufs=1) as cpool,
            tc.tile_pool(name="xin", bufs=4) as xpool,
            tc.tile_pool(name="xtsb", bufs=4) as xtpool,
            tc.tile_pool(name="thsb", bufs=4) as thpool,
            tc.tile_pool(name="attnsb", bufs=4) as apool,
            tc.tile_pool(name="ssb", bufs=6) as spool,
            tc.tile_pool(name="outsb", bufs=3) as opool,
            tc.tile_pool(name="xtps", bufs=2, space="PSUM") as xtps_pool,
            tc.tile_pool(name="htps", bufs=2, space="PSUM") as htps_pool,
            tc.tile_pool(name="atps", bufs=2, space="PSUM") as atps_pool,
            tc.tile_pool(name="accps", bufs=2, space="PSUM") as accps_pool,
        ):
            # constants: loaded once, before the (optional) repeat loop
            cst_sb = cpool.tile([P, n_const], F32, name="cst_sb")
            nc.sync.dma_start(out=cst_sb, in_=cst_d)
            o = 0
            b1_sb = cst_sb[:, o:o + 1]; o += 1
            iota_sb = cst_sb[:, o:o + G_WIN]; o += G_WIN
            relT_sb = cst_sb[:, o:o + n_tiles]; o += n_tiles

            c16_sb = cpool.tile([P, n_const16], F16, name="c16_sb")
            nc.sync.dma_start(out=c16_sb, in_=c16_d)
            w1h = [c16_sb[:, 0:P], c16_sb[:, P:2 * P]]
            idn_sb = c16_sb[:, 2 * P:3 * P]
            w2_sb = c16_sb[:, 3 * P:3 * P + 1]

            def body():
                for w in range(n_wins):
                    t0 = w * win_tiles
                    wt = win_tiles

                    x_chunk = xpool.tile([P, wt * D_IN], F16, name="x_chunk",
                                         tag="x_chunk")
                    nc.sync.dma_start(
                        out=x_chunk, in_=x_d[w * P:(w + 1) * P, :])

                    acc_ps = accps_pool.tile([G_WIN, D_IN], F32, name="acc_ps",
                                             tag="acc_ps")

                    groups = [tuple(range(g, min(g + 2, wt)))
                              for g in range(0, wt, 2)]
                    for gi, grp in enumerate(groups):
                        ng = len(grp)
                        if stages == "noattn":
                            for i, lt in enumerate(grp):
                                gt = t0 + lt
                                s_sb = spool.tile([P, G_WIN], F16,
                                                  name="s_sb", tag="s_sb")
                                nc.vector.tensor_scalar(
                                    s_sb, iota_sb, relT_sb[:, gt:gt + 1],
                                    scalar2=None,
                                    op0=mybir.AluOpType.is_equal)
                                nc.tensor.matmul(
                                    acc_ps, s_sb,
                                    x_chunk[:, lt * D_IN:(lt + 1) * D_IN],
                                    start=(lt == 0), stop=(lt == wt - 1))
                            continue
                        # --- xT via regular matmul: xT_half = x_half.T @ I ---
                        xt_ps = xtps_pool.tile([P, ng * D_IN], F32,
                                               name="xt_ps", tag="xt_ps")
                        for i, lt in enumerate(grp):
                            for q in range(2):
                                nc.tensor.matmul(
                                    xt_ps[:, i * D_IN + q * P:
                                          i * D_IN + (q + 1) * P],
                                    x_chunk[:, lt * D_IN + q * P:
                                            lt * D_IN + (q + 1) * P],
                                    idn_sb, start=True, stop=True)
                        # PSUM -> SBUF fp16, always DVE (single reader per
                        # buffer keeps PE within its 2-engine wait limit;
                        # warm ACT copies are ~2.1x slower than DVE and ACT
                        # is reserved for tanh)
                        xt_sb = xtpool.tile([P, ng * D_IN], F16, name="xt_sb",
                                            tag="xt_sb")
                        nc.vector.tensor_copy(xt_sb, xt_ps[:, 0:ng * D_IN])

                        # --- hT accumulated over the two d-halves ---
                        ht_ps = htps_pool.tile([P, ng * D_ATT], F32,
                                               name="ht_ps", tag="ht_ps")
                        xt4 = xt_sb.rearrange("p (t h n) -> p t h n",
                                              t=ng, h=2)
                        ht3 = ht_ps.rearrange("p (t n) -> p t n", t=ng)
                        nc.tensor.matmul(ht3, w1h[0], xt4[:, :, 0, :],
                                         start=True, stop=False)
                        nc.tensor.matmul(ht3, w1h[1], xt4[:, :, 1, :],
                                         start=False, stop=True)

                        # --- th = tanh(hT + b1), fp16 out ---
                        th_sb = thpool.tile([P, ng * D_ATT], F16, name="th_sb",
                                            tag="th_sb")
                        nc.scalar.activation(th_sb, ht_ps[:, 0:ng * D_ATT],
                                             mybir.ActivationFunctionType.Tanh,
                                             bias=b1_sb, scale=1.0)

                        # --- attn[n] = th.T @ W2 (fp16, free dim 1) ---
                        at_ps = atps_pool.tile([P, ng], F32, name="at_ps",
                                               tag="at_ps")
                        for i in range(ng):
                            nc.tensor.matmul(
                                at_ps[:, i:i + 1],
                                th_sb[:, i * D_ATT:(i + 1) * D_ATT],
                                w2_sb, start=True, stop=True)
                        # +b2 on ACT (Copy activation with constant bias):
                        # keeps DVE free for the xt copies and S-builds, and
                        # gives at_ps a single reader engine.
                        at_sb = apool.tile([P, ng], F32, name="at_sb",
                                           tag="at_sb")
                        nc.scalar.activation(at_sb, at_ps[:, 0:ng],
                                             mybir.ActivationFunctionType.Copy,
                                             bias=float(b2), scale=1.0)

                        # --- S = (iota == rel) * attn' ; acc += S.T @ x ---
                        # (S-build stays on DVE: GpSimd shares an SBUF port
                        # with DVE and its per-instruction overhead regressed
                        # the kernel ~2x when tried.)
                        for i, lt in enumerate(grp):
                            gt = t0 + lt
                            s_sb = spool.tile([P, G_WIN], F16, name="s_sb",
                                              tag="s_sb")
                            nc.vector.tensor_scalar(
                                s_sb, iota_sb, relT_sb[:, gt:gt + 1],
                                at_sb[:, i:i + 1],
                                mybir.AluOpType.is_equal, mybir.AluOpType.mult)
                            if stages == "nopool":
                                if lt == 0:
                                    nc.tensor.matmul(
                                        acc_ps, s_sb,
                                        x_chunk[:, 0:D_IN],
                                        start=True, stop=True)
                            else:
                                nc.tensor.matmul(
                                    acc_ps, s_sb,
                                    x_chunk[:, lt * D_IN:(lt + 1) * D_IN],
                                    start=(lt == 0), stop=(lt == wt - 1))

                    # --- flush window accumulator (DVE) ---
                    out_sb = opool.tile([G_WIN, D_IN], F32, name="out_sb",
                                        tag="out_sb")
                    nc.vector.tensor_copy(out_sb, acc_ps)
                    nc.sync.dma_start(
                        out=out_d[w * G_WIN:(w + 1) * G_WIN, :], in_=out_sb)

            if reps == 1:
                body()
            else:
                # body is ~4k instructions (>> one IRAM block): arm the
                # branch prefetchers so the back-edge I$-hits.
                with tc.For_i(0, reps, 1,
                              hint_engines=(mybir.EngineType.PE,
                                            mybir.EngineType.DVE,
                                            mybir.EngineType.Activation)):
                    body()

    nc.compile()
    return nc


def choose_win_tiles(batch_slices, n_tiles):
    """Pick the biggest window size (in tiles) such that every window of
    every core spans < G_WIN distinct graphs (batch is sorted, so the span
    is last - first + 1)."""
    for wt in (16, 8, 4, 2, 1):
        ok = True
        for bc in batch_slices:
            nn = len(bc)
            for s in range(0, nn, wt * P):
                e = min(nn, s + wt * P)
                if bc[e - 1] - bc[s] + 1 > G_WIN - 1:
                    ok = False
                    break
            if not ok:
                break
        if ok:
            return wt
    return 1


def prep_core(x_real, batch_real, n_tiles, win_tiles):
    """Pad one core's slice to n_tiles*128 nodes (whole windows), cast to
    fp16, swizzle per window to a partition-contiguous layout, and build
    relT + g0s.

    Returns (x_sw [n_wins*128, win_tiles*256] f16, relT [128, n_tiles] f32,
    g0s). Padded nodes get rel = -1 so they never match the one-hot iota.
    x_sw[w*128 + p, t*256:(t+1)*256] = x[(w*win_tiles + t)*128 + p].
    """
    assert n_tiles % win_tiles == 0
    npad = n_tiles * P
    n_real = x_real.shape[0]
    assert n_real <= npad
    x_pad = np.zeros((npad, D_IN), dtype=np.float16)
    x_pad[:n_real] = x_real.astype(np.float16)
    b = np.full(npad, -1, dtype=np.int64)
    b[:n_real] = batch_real

    n_wins = n_tiles // win_tiles
    x_sw = np.ascontiguousarray(
        x_pad.reshape(n_wins, win_tiles, P, D_IN).transpose(0, 2, 1, 3)
    ).reshape(n_wins * P, win_tiles * D_IN)

    rel = np.full(npad, -1.0, dtype=np.float32)
    g0s = np.zeros(n_wins, dtype=np.int64)
    for w in range(n_wins):
        s = w * win_tiles * P
        e = (w + 1) * win_tiles * P
        seg = b[s:e]
        realm = seg >= 0
        g0 = int(seg[realm][0]) if realm.any() else 0
        g0s[w] = g0
        rw = (seg - g0).astype(np.float32)
        rw[~realm] = -1.0
        assert rw.max() < G_WIN, (
            f"window spans too many graphs: {rw.max()} >= {G_WIN}")
        rel[s:e] = rw
    relT = np.ascontiguousarray(rel.reshape(n_tiles, P).T)
    return x_sw, relT, g0s


def make_consts(W1, b1, W2):
    """Returns (cst_f32 [128, 33], cst16 [128, 385])."""
    W1 = np.asarray(W1, dtype=np.float32)
    cst = np.ascontiguousarray(np.concatenate([
        np.asarray(b1, np.float32).reshape(P, 1),
        np.broadcast_to(np.arange(G_WIN, dtype=np.float32), (P, G_WIN)),
    ], axis=1))
    w1h = W1.astype(np.float16)
    cst16 = np.ascontiguousarray(np.concatenate([
        w1h[0:P, :], w1h[P:2 * P, :],
        np.eye(P, dtype=np.float16),
        np.asarray(W2, np.float16).reshape(P, 1),
    ], axis=1))
    return cst, cst16


def postprocess(raws, g0s_per_core, num_graphs):
    """raws: per-core [n_wins*G_WIN, D_IN] raw window sums -> [G, D_IN]."""
    out = np.zeros((num_graphs, D_IN), dtype=np.float64)
    for raw, g0s in zip(raws, g0s_per_core):
        raw3 = raw.reshape(-1, G_WIN, D_IN)
        for w, g0 in enumerate(g0s):
            width = min(G_WIN, num_graphs - int(g0))
            out[g0:g0 + width] += raw3[w, :width]
    return out.astype(np.float32)


def prepare(x, batch, num_graphs, W1, b1, W2, b2, reps=1):
    """Host-side prep: shard, window metadata, and the Bass program.

    Returns (nc, in_maps, g0s_per_core, num_graphs).
    """
    x = np.asarray(x, dtype=np.float32)
    batch = np.asarray(batch).astype(np.int64)
    num_graphs = int(num_graphs)
    W1 = np.asarray(W1, dtype=np.float32)
    b1 = np.asarray(b1, dtype=np.float32)
    W2 = np.asarray(W2, dtype=np.float32)
    b2f = float(np.asarray(b2).reshape(-1)[0])

    n = x.shape[0]
    assert n == N_NODES and x.shape[1] == D_IN
    assert np.all(np.diff(batch) >= 0), "batch must be sorted"

    bounds = [(c * NODES_PER_CORE,
               min(n, (c + 1) * NODES_PER_CORE) if c < N_CORES - 1 else n)
              for c in range(N_CORES)]

    wt = choose_win_tiles([batch[s:e] for s, e in bounds], TILES_PER_CORE)
    n_tiles_pad = math.ceil(TILES_PER_CORE / wt) * wt

    cbase, cst16 = make_consts(W1, b1, W2)
    in_maps = []
    g0s_per_core = []
    for s, e in bounds:
        x_sw, relT, g0s = prep_core(x[s:e], batch[s:e], n_tiles_pad, wt)
        cst = np.ascontiguousarray(np.concatenate([cbase, relT], axis=1))
        in_maps.append({"x16": x_sw, "cst": cst, "cst16": cst16})
        g0s_per_core.append(g0s)
    nc = build_program(n_tiles_pad, wt, b2f, reps=reps)
    return nc, in_maps, g0s_per_core, num_graphs


def kernel(x, batch, num_graphs, W1, b1, W2, b2):
    nc, in_maps, g0s_per_core, num_graphs = prepare(
        x, batch, num_graphs, W1, b1, W2, b2)
    res = bass_utils.run_bass_kernel_spmd(
        nc, in_maps, core_ids=list(range(N_CORES)))
    raws = [r["out"] for r in res.results]
    return postprocess(raws, g0s_per_core, num_graphs)


# revision 6
# speedup vs baseline: 2.3192x; 1.0698x over previous
"""Trainium2 Bass kernel: AttentionPooling (attention-weighted global_add_pool).

Computes, for x [N, 256], sorted graph ids batch [N] (num_graphs=4096):
    h    = tanh(x @ W1 + b1)            # [N, 128]
    attn = h @ W2 + b2                  # [N, 1]
    out  = segment_sum(x * attn, batch) # [4096, 256]

v2 design (vs the fp16-compensated v1):
  - Pure fp16 inputs (x, W1, W2 as fp16; f32 PSUM accumulation). Host-checked
    rel err ~5e-4, well under the 2e-2 gate. Halves HBM traffic (32 MB/core)
    and removes 4 of the 10 matmul terms.
  - Transposes are REGULAR matmuls against a stationary identity
    (out = x_half.T @ I). PE-transpose-mode ops cost ~275 ns each (SBUF
    access latency dominated, don't keep HAM warm); a regular fp16 matmul
    streams N=128 rows at ~81 ns in a production stream and keeps the PE
    clock warm.
  - tanh output and the attention dot run in fp16 (th stationary, w2 fp16).
  - Optional reps>1 wraps the whole body in a hardware For_i loop: the NEFF
    re-executes the complete kernel (x re-read from HBM each iteration)
    reps times. Used by the harness to measure per-execution device time
    differentially through the high-overhead axon tunnel.

Per 128-row tile on-device:
  - xT halves via 2 regular matmuls (lhsT = x_half, rhs = I) -> f32 PSUM
  - PSUM -> SBUF fp16 copy (DVE/ACT alternating per group: single reader
    engine per PSUM buffer keeps every PE instruction within the
    2-engine sync-wait limit)
  - hT[a, n] = sum_d W1[d, a] xT[d, n]  (two fp16 K=128 matmuls, PSUM acc)
  - th = tanh(hT + b1) on ScalarE, fp16 out (bias per-partition: a)
  - attn[n, 1] = th.T @ W2 (fp16 matmul, free dim 1)
  - S[n, j] = (iota == rel[n]) * (attn[n] + b2)  (one fused DVE tensor_scalar
    into fp16; rel = batch - first_graph_of_window, host-precomputed)
  - acc[j, d] += S.T @ x_tile  (f32 PSUM accumulation across the window)
Window accumulators [32, 256] flush raw to DRAM; the host maps window slot
j -> graph g0[w] + j and sums across windows/cores (~8 MB, cheap).
"""

import math

import numpy as np

import concourse.bass as bass
import concourse.mybir as mybir
import concourse.tile as tile
from concourse import bacc, bass_utils

P = 128
D_IN = 256
D_ATT = 128
G_WIN = 32  # one-hot width = max graphs a window may span

N_NODES = 500_000
NUM_GRAPHS = 4096
N_CORES = 8
NODES_PER_CORE = N_NODES // N_CORES  # 62500
TILES_PER_CORE = math.ceil(NODES_PER_CORE / P)  # 489

F32 = mybir.dt.float32
F16 = mybir.dt.float16


def build_program(n_tiles: int, win_tiles: int, b2: float, reps: int = 1,
                  stages: str = "all"):
    """Build the single-core Bass program (same NEFF runs SPMD on all cores).

    reps > 1 wraps the body in a hardware For_i loop for differential
    device-time measurement; every iteration re-reads x from HBM and
    rewrites the full output.

    stages: "all" (the real kernel) | "noattn" (skip transpose/hT/tanh/attn;
    S = bare one-hot) | "nopool" (only one pool matmul per window) —
    timing-only ablations for bottleneck attribution; their outputs are
    numerically wrong."""
    assert n_tiles % win_tiles == 0, "pad tiles to a whole number of windows"
    n_wins = n_tiles // win_tiles
    nc = bacc.Bacc(trn_type="TRN2", target_bir_lowering=False, debug=False,
                   num_devices=N_CORES)

    n_const = 1 + G_WIN + n_tiles                 # b1 | iota | relT
    n_const16 = 2 * D_ATT + P + 1                 # W1 halves | idn | w2
    # x16: per window [128, win_tiles*256] fp16, host-swizzled so each
    # window's DMA is partition-contiguous (8 KB/partition, 128 descriptors)
    x_d = nc.dram_tensor("x16", [n_wins * P, win_tiles * D_IN], F16,
                         kind="ExternalInput").ap()
    cst_d = nc.dram_tensor("cst", [P, n_const], F32, kind="ExternalInput").ap()
    c16_d = nc.dram_tensor("cst16", [P, n_const16], F16,
                           kind="ExternalInput").ap()
    out_d = nc.dram_tensor("out", [n_wins * G_WIN, D_IN], F32,
                           kind="ExternalOutput").ap()

    with tile.TileContext(nc) as tc:
        with (
            tc.tile_pool(name="consts", bufs=1) as cpool,
            tc.tile_pool(name="xin", bufs=4) as xpool,
            tc.tile_pool(name="xtsb", bufs=4) as xtpool,
            tc.tile_pool(name="thsb", bufs=4) as thpool,
            tc.tile_pool(name="attnsb", bufs=4) as apool,
            tc.tile_pool(name="ssb", bufs=6) as spool,
            tc.tile_pool(name="outsb", bufs=3) as opool,
            tc.tile_pool(name="xtps", bufs=2, space="PSUM") as xtps_pool,
            tc.tile_pool(name="htps", bufs=2, space="PSUM") as htps_pool,
            tc.tile_pool(name="atps", bufs=2, space="PSUM") as atps_pool,
            tc.tile_pool(name="accps", bufs=2, space="PSUM") as accps_pool,
        ):
            # constants: loaded once, before the (optional) repeat loop
            cst_sb = cpool.tile([P, n_const], F32, name="cst_sb")
            nc.sync.dma_start(out=cst_sb, in_=cst_d)
            o = 0
            b1_sb = cst_sb[:, o:o + 1]; o += 1
            iota_sb = cst_sb[:, o:o + G_WIN]; o += G_WIN
            relT_sb = cst_sb[:, o:o + n_tiles]; o += n_tiles

            c16_sb = cpool.tile([P, n_const16], F16, name="c16_sb")
            nc.sync.dma_start(out=c16_sb, in_=c16_d)
            w1h = [c16_sb[:, 0:P], c16_sb[:, P:2 * P]]
            idn_sb = c16_sb[:, 2 * P:3 * P]
            w2_sb = c16_sb[:, 3 * P:3 * P + 1]

            wt = win_tiles
            n_groups_per_win = (wt + 1) // 2
            all_groups = []
            for w in range(n_wins):
                for g0 in range(0, wt, 2):
                    all_groups.append(
                        (w, tuple(range(g0, min(g0 + 2, wt)))))

            # pipeline state, keyed by flat group index
            st: dict[int, dict] = {}
            win_state: dict[int, dict] = {}

            def get_win(w):
                """Allocate per-window tiles lazily (x chunk at stage A,
                accumulator at stage D)."""
                if w not in win_state:
                    win_state[w] = {}
                return win_state[w]

            def stage_a(k):
                """DMA (first group of window) + transposes + PSUM->SBUF."""
                w, grp = all_groups[k]
                ng = len(grp)
                ws = get_win(w)
                if "x_chunk" not in ws:
                    x_chunk = xpool.tile([P, wt * D_IN], F16, name="x_chunk",
                                         tag="x_chunk")
                    nc.sync.dma_start(
                        out=x_chunk, in_=x_d[w * P:(w + 1) * P, :])
                    ws["x_chunk"] = x_chunk
                x_chunk = ws["x_chunk"]
                xt_ps = xtps_pool.tile([P, ng * D_IN], F32,
                                       name="xt_ps", tag="xt_ps")
                for i, lt in enumerate(grp):
                    for q in range(2):
                        nc.tensor.matmul(
                            xt_ps[:, i * D_IN + q * P:i * D_IN + (q + 1) * P],
                            x_chunk[:, lt * D_IN + q * P:
                                    lt * D_IN + (q + 1) * P],
                            idn_sb, start=True, stop=True)
                xt_sb = xtpool.tile([P, ng * D_IN], F16, name="xt_sb",
                                    tag="xt_sb")
                nc.vector.tensor_copy(xt_sb, xt_ps[:, 0:ng * D_IN])
                st[k] = {"xt_sb": xt_sb, "ng": ng}

            def stage_b(k):
                """hT matmuls + tanh."""
                ng = st[k]["ng"]
                ht_ps = htps_pool.tile([P, ng * D_ATT], F32,
                                       name="ht_ps", tag="ht_ps")
                xt4 = st[k]["xt_sb"].rearrange("p (t h n) -> p t h n",
                                               t=ng, h=2)
                ht3 = ht_ps.rearrange("p (t n) -> p t n", t=ng)
                nc.tensor.matmul(ht3, w1h[0], xt4[:, :, 0, :],
                                 start=True, stop=False)
                nc.tensor.matmul(ht3, w1h[1], xt4[:, :, 1, :],
                                 start=False, stop=True)
                th_sb = thpool.tile([P, ng * D_ATT], F16, name="th_sb",
                                    tag="th_sb")
                nc.scalar.activation(th_sb, ht_ps[:, 0:ng * D_ATT],
                                     mybir.ActivationFunctionType.Tanh,
                                     bias=b1_sb, scale=1.0)
                st[k]["th_sb"] = th_sb

            def stage_c(k):
                """attn matmuls + bias + S one-hots."""
                w, grp = all_groups[k]
                ng = st[k]["ng"]
                th_sb = st[k]["th_sb"]
                at_ps = atps_pool.tile([P, ng], F32, name="at_ps",
                                       tag="at_ps")
                for i in range(ng):
                    nc.tensor.matmul(at_ps[:, i:i + 1],
                                     th_sb[:, i * D_ATT:(i + 1) * D_ATT],
                                     w2_sb, start=True, stop=True)
                at_sb = apool.tile([P, ng], F32, name="at_sb", tag="at_sb")
                nc.scalar.activation(at_sb, at_ps[:, 0:ng],
                                     mybir.ActivationFunctionType.Copy,
                                     bias=float(b2), scale=1.0)
                s_tiles = []
                for i, lt in enumerate(grp):
                    gt = w * wt + lt
                    s_sb = spool.tile([P, G_WIN], F16, name="s_sb",
                                      tag="s_sb")
                    nc.vector.tensor_scalar(
                        s_sb, iota_sb, relT_sb[:, gt:gt + 1],
                        at_sb[:, i:i + 1],
                        mybir.AluOpType.is_equal, mybir.AluOpType.mult)
                    s_tiles.append(s_sb)
                st[k]["s_tiles"] = s_tiles

            def stage_d(k):
                """pool matmuls into the window accumulator + flush."""
                w, grp = all_groups[k]
                ws = get_win(w)
                if "acc_ps" not in ws:
                    ws["acc_ps"] = accps_pool.tile([G_WIN, D_IN], F32,
                                                   name="acc_ps",
                                                   tag="acc_ps")
                acc_ps = ws["acc_ps"]
                x_chunk = ws["x_chunk"]
                for i, lt in enumerate(grp):
                    nc.tensor.matmul(
                        acc_ps, st[k]["s_tiles"][i],
                        x_chunk[:, lt * D_IN:(lt + 1) * D_IN],
                        start=(lt == 0), stop=(lt == wt - 1))
                if grp[-1] == wt - 1:
                    out_sb = opool.tile([G_WIN, D_IN], F32, name="out_sb",
                                        tag="out_sb")
                    nc.vector.tensor_copy(out_sb, acc_ps)
                    nc.sync.dma_start(
                        out=out_d[w * G_WIN:(w + 1) * G_WIN, :], in_=out_sb)
                    del win_state[w]
                del st[k]

            def body():
                st.clear()
                win_state.clear()
                ntot = len(all_groups)
                for k in range(ntot + 3):
                    if k < ntot:
                        stage_a(k)
                    if 1 <= k < ntot + 1:
                        stage_b(k - 1)
                    if 2 <= k < ntot + 2:
                        stage_c(k - 2)
                    if 3 <= k < ntot + 3:
                        stage_d(k - 3)

            def body_ablation():
                for w in range(n_wins):
                    t0 = w * wt

                    x_chunk = xpool.tile([P, wt * D_IN], F16, name="x_chunk",
                                         tag="x_chunk")
                    nc.sync.dma_start(
                        out=x_chunk, in_=x_d[w * P:(w + 1) * P, :])

                    acc_ps = accps_pool.tile([G_WIN, D_IN], F32, name="acc_ps",
                                             tag="acc_ps")

                    groups = [tuple(range(g, min(g + 2, wt)))
                              for g in range(0, wt, 2)]
                    for gi, grp in enumerate(groups):
                        ng = len(grp)
                        if stages == "noattn":
                            for i, lt in enumerate(grp):
                                gt = t0 + lt
                                s_sb = spool.tile([P, G_WIN], F16,
                                                  name="s_sb", tag="s_sb")
                                nc.vector.tensor_scalar(
                                    s_sb, iota_sb, relT_sb[:, gt:gt + 1],
                                    scalar2=None,
                                    op0=mybir.AluOpType.is_equal)
                                nc.tensor.matmul(
                                    acc_ps, s_sb,
                                    x_chunk[:, lt * D_IN:(lt + 1) * D_IN],
                                    start=(lt == 0), stop=(lt == wt - 1))
                            continue
                        # --- xT via regular matmul: xT_half = x_half.T @ I ---
                        xt_ps = xtps_pool.tile([P, ng * D_IN], F32,
                                               name="xt_ps", tag="xt_ps")
                        for i, lt in enumerate(grp):
                            for q in range(2):
                                nc.tensor.matmul(
                                    xt_ps[:, i * D_IN + q * P:
                                          i * D_IN + (q + 1) * P],
                                    x_chunk[:, lt * D_IN + q * P:
                                            lt * D_IN + (q + 1) * P],
                                    idn_sb, start=True, stop=True)
                        # PSUM -> SBUF fp16, always DVE (single reader per
                        # buffer keeps PE within its 2-engine wait limit;
                        # warm ACT copies are ~2.1x slower than DVE and ACT
                        # is reserved for tanh)
                        xt_sb = xtpool.tile([P, ng * D_IN], F16, name="xt_sb",
                                            tag="xt_sb")
                        nc.vector.tensor_copy(xt_sb, xt_ps[:, 0:ng * D_IN])

                        # --- hT accumulated over the two d-halves ---
                        ht_ps = htps_pool.tile([P, ng * D_ATT], F32,
                                               name="ht_ps", tag="ht_ps")
                        xt4 = xt_sb.rearrange("p (t h n) -> p t h n",
                                              t=ng, h=2)
                        ht3 = ht_ps.rearrange("p (t n) -> p t n", t=ng)
                        nc.tensor.matmul(ht3, w1h[0], xt4[:, :, 0, :],
                                         start=True, stop=False)
                        nc.tensor.matmul(ht3, w1h[1], xt4[:, :, 1, :],
                                         start=False, stop=True)

                        # --- th = tanh(hT + b1), fp16 out ---
                        th_sb = thpool.tile([P, ng * D_ATT], F16, name="th_sb",
                                            tag="th_sb")
                        nc.scalar.activation(th_sb, ht_ps[:, 0:ng * D_ATT],
                                             mybir.ActivationFunctionType.Tanh,
                                             bias=b1_sb, scale=1.0)

                        # --- attn[n] = th.T @ W2 (fp16, free dim 1) ---
                        at_ps = atps_pool.tile([P, ng], F32, name="at_ps",
                                               tag="at_ps")
                        for i in range(ng):
                            nc.tensor.matmul(
                                at_ps[:, i:i + 1],
                                th_sb[:, i * D_ATT:(i + 1) * D_ATT],
                                w2_sb, start=True, stop=True)
                        # +b2 on ACT (Copy activation with constant bias):
                        # keeps DVE free for the xt copies and S-builds, and
                        # gives at_ps a single reader engine.
                        at_sb = apool.tile([P, ng], F32, name="at_sb",
                                           tag="at_sb")
                        nc.scalar.activation(at_sb, at_ps[:, 0:ng],
                                             mybir.ActivationFunctionType.Copy,
                                             bias=float(b2), scale=1.0)

                        # --- S = (iota == rel) * attn' ; acc += S.T @ x ---
                        # (S-build stays on DVE: GpSimd shares an SBUF port
                        # with DVE and its per-instruction overhead regressed
                        # the kernel ~2x when tried.)
                        for i, lt in enumerate(grp):
                            gt = t0 + lt
                            s_sb = spool.tile([P, G_WIN], F16, name="s_sb",
                                              tag="s_sb")
                            nc.vector.tensor_scalar(
                                s_sb, iota_sb, relT_sb[:, gt:gt + 1],
                                at_sb[:, i:i + 1],
                                mybir.AluOpType.is_equal, mybir.AluOpType.mult)
                            if stages == "nopool":
                                if lt == 0:
                                    nc.tensor.matmul(
                                        acc_ps, s_sb,
                                        x_chunk[:, 0:D_IN],
                                        start=True, stop=True)
                            else:
                                nc.tensor.matmul(
                                    acc_ps, s_sb,
                                    x_chunk[:, lt * D_IN:(lt + 1) * D_IN],
                                    start=(lt == 0), stop=(lt == wt - 1))

                    # --- flush window accumulator (DVE) ---
                    out_sb = opool.tile([G_WIN, D_IN], F32, name="out_sb",
                                        tag="out_sb")
                    nc.vector.tensor_copy(out_sb, acc_ps)
                    nc.sync.dma_start(
                        out=out_d[w * G_WIN:(w + 1) * G_WIN, :], in_=out_sb)

            run = body if stages == "all" else body_ablation
            if reps == 1:
                run()
            else:
                # body is ~4k instructions (>> one IRAM block): arm the
                # branch prefetchers so the back-edge I$-hits.
                with tc.For_i(0, reps, 1,
                              hint_engines=(mybir.EngineType.PE,
                                            mybir.EngineType.DVE,
                                            mybir.EngineType.Activation)):
                    run()

    nc.compile()
    return nc


def choose_win_tiles(batch_slices, n_tiles):
    """Pick the biggest window size (in tiles) such that every window of
    every core spans < G_WIN distinct graphs (batch is sorted, so the span
    is last - first + 1)."""
    for wt in (16, 8, 4, 2, 1):
        ok = True
        for bc in batch_slices:
            nn = len(bc)
            for s in range(0, nn, wt * P):
                e = min(nn, s + wt * P)
                if bc[e - 1] - bc[s] + 1 > G_WIN - 1:
                    ok = False
                    break
            if not ok:
                break
        if ok:
            return wt
    return 1


def prep_core(x_real, batch_real, n_tiles, win_tiles):
    """Pad one core's slice to n_tiles*128 nodes (whole windows), cast to
    fp16, swizzle per window to a partition-contiguous layout, and build
    relT + g0s.

    Returns (x_sw [n_wins*128, win_tiles*256] f16, relT [128, n_tiles] f32,
    g0s). Padded nodes get rel = -1 so they never match the one-hot iota.
    x_sw[w*128 + p, t*256:(t+1)*256] = x[(w*win_tiles + t)*128 + p].
    """
    assert n_tiles % win_tiles == 0
    npad = n_tiles * P
    n_real = x_real.shape[0]
    assert n_real <= npad
    x_pad = np.zeros((npad, D_IN), dtype=np.float16)
    x_pad[:n_real] = x_real.astype(np.float16)
    b = np.full(npad, -1, dtype=np.int64)
    b[:n_real] = batch_real

    n_wins = n_tiles // win_tiles
    x_sw = np.ascontiguousarray(
        x_pad.reshape(n_wins, win_tiles, P, D_IN).transpose(0, 2, 1, 3)
    ).reshape(n_wins * P, win_tiles * D_IN)

    rel = np.full(npad, -1.0, dtype=np.float32)
    g0s = np.zeros(n_wins, dtype=np.int64)
    for w in range(n_wins):
        s = w * win_tiles * P
        e = (w + 1) * win_tiles * P
        seg = b[s:e]
        realm = seg >= 0
        g0 = int(seg[realm][0]) if realm.any() else 0
        g0s[w] = g0
        rw = (seg - g0).astype(np.float32)
        rw[~realm] = -1.0
        assert rw.max() < G_WIN, (
            f"window spans too many graphs: {rw.max()} >= {G_WIN}")
        rel[s:e] = rw
    relT = np.ascontiguousarray(rel.reshape(n_tiles, P).T)
    return x_sw, relT, g0s


def make_consts(W1, b1, W2):
    """Returns (cst_f32 [128, 33], cst16 [128, 385])."""
    W1 = np.asarray(W1, dtype=np.float32)
    cst = np.ascontiguousarray(np.concatenate([
        np.asarray(b1, np.float32).reshape(P, 1),
        np.broadcast_to(np.arange(G_WIN, dtype=np.float32), (P, G_WIN)),
    ], axis=1))
    w1h = W1.astype(np.float16)
    cst16 = np.ascontiguousarray(np.concatenate([
        w1h[0:P, :], w1h[P:2 * P, :],
        np.eye(P, dtype=np.float16),
        np.asarray(W2, np.float16).reshape(P, 1),
    ], axis=1))
    return cst, cst16


def postprocess(raws, g0s_per_core, num_graphs):
    """raws: per-core [n_wins*G_WIN, D_IN] raw window sums -> [G, D_IN]."""
    out = np.zeros((num_graphs, D_IN), dtype=np.float64)
    for raw, g0s in zip(raws, g0s_per_core):
        raw3 = raw.reshape(-1, G_WIN, D_IN)
        for w, g0 in enumerate(g0s):
            width = min(G_WIN, num_graphs - int(g0))
            out[g0:g0 + width] += raw3[w, :width]
    return out.astype(np.float32)


def prepare(x, batch, num_graphs, W1, b1, W2, b2, reps=1):
    """Host-side prep: shard, window metadata, and the Bass program.

    Returns (nc, in_maps, g0s_per_core, num_graphs).
    """
    x = np.asarray(x, dtype=np.float32)
    batch = np.asarray(batch).astype(np.int64)
    num_graphs = int(num_graphs)
    W1 = np.asarray(W1, dtype=np.float32)
    b1 = np.asarray(b1, dtype=np.float32)
    W2 = np.asarray(W2, dtype=np.float32)
    b2f = float(np.asarray(b2).reshape(-1)[0])

    n = x.shape[0]
    assert n == N_NODES and x.shape[1] == D_IN
    assert np.all(np.diff(batch) >= 0), "batch must be sorted"

    bounds = [(c * NODES_PER_CORE,
               min(n, (c + 1) * NODES_PER_CORE) if c < N_CORES - 1 else n)
              for c in range(N_CORES)]

    wt = choose_win_tiles([batch[s:e] for s, e in bounds], TILES_PER_CORE)
    n_tiles_pad = math.ceil(TILES_PER_CORE / wt) * wt

    cbase, cst16 = make_consts(W1, b1, W2)
    in_maps = []
    g0s_per_core = []
    for s, e in bounds:
        x_sw, relT, g0s = prep_core(x[s:e], batch[s:e], n_tiles_pad, wt)
        cst = np.ascontiguousarray(np.concatenate([cbase, relT], axis=1))
        in_maps.append({"x16": x_sw, "cst": cst, "cst16": cst16})
        g0s_per_core.append(g0s)
    nc = build_program(n_tiles_pad, wt, b2f, reps=reps)
    return nc, in_maps, g0s_per_core, num_graphs


def kernel(x, batch, num_graphs, W1, b1, W2, b2):
    nc, in_maps, g0s_per_core, num_graphs = prepare(
        x, batch, num_graphs, W1, b1, W2, b2)
    res = bass_utils.run_bass_kernel_spmd(
        nc, in_maps, core_ids=list(range(N_CORES)))
    raws = [r["out"] for r in res.results]
    return postprocess(raws, g0s_per_core, num_graphs)
